# revision 4
# baseline (speedup 1.0000x reference)
"""Trainium2 Bass kernel for nn_Attention_55645596287359 (sparse_attention).

Math per (b, h) pair (T=512, N=8192, D=256):
    QR   = rope(Q)                                   # interleaved-pair rope
    S    = (QR @ QR^T) * SCALE, strictly-causal-masked (j < i)
    out  = S @ V + QR @ state
    newS = state + (QR^T @ V) * SCALE

Sharding: 16 (b, h) pairs over 8 cores, 2 pairs per core (pure batch dims,
no cross-core communication).  Each core streams cos/sin once, ropes both
pairs into SBUF-resident bf16 QR tiles ([t, n] layout), then runs a fused
n-tile loop per pair: PE-transpose to get QR^T slabs, score accumulation,
out accumulation (QR @ state), and chunk-state (QR^T @ V') with the state
addition folded in as an identity matmul.  f32 in HBM, bf16 on the
TensorEngine, f32 PSUM accumulation.
"""

import sys

sys.path.insert(0, "/opt/trn_rl_repo")

import numpy as np
import ml_dtypes

B, NH, T, N, D = 4, 4, 512, 8192, 256
SCALE = float(N) ** -0.5
NCORES = 8
C = 1024              # rope streaming chunk (free-dim elements)
NC8 = N // C          # 8 chunks
NT = N // 128         # 64 n-tiles
TTN = T // 128        # 4 t-tiles
SB = 8                # state/new-state DMA batch (n-tiles per DMA)

F32 = None
BF16 = None

_CACHE = {}


def _build():
    import concourse.bacc as bacc
    import concourse.mybir as mybir
    import concourse.tile as tile
    from concourse.alu_op_type import AluOpType
    from contextlib import ExitStack

    F32 = mybir.dt.float32
    BF16 = mybir.dt.bfloat16

    nc = bacc.Bacc("TRN2", target_bir_lowering=False, debug=False)

    q_d = nc.dram_tensor("q", [2, T, N], F32, kind="ExternalInput").ap()
    v_d = nc.dram_tensor("v", [T, D], F32, kind="ExternalInput").ap()
    st_d = nc.dram_tensor("state_in", [2, N, D], F32, kind="ExternalInput").ap()
    cos_d = nc.dram_tensor("cos", [T, N], F32, kind="ExternalInput").ap()
    sin_d = nc.dram_tensor("sin", [T, N], F32, kind="ExternalInput").ap()
    mask_d = nc.dram_tensor("mask", [128, 128], F32, kind="ExternalInput").ap()
    id_d = nc.dram_tensor("ident", [128, 128], BF16, kind="ExternalInput").ap()
    out_d = nc.dram_tensor("out", [2, T, D], F32, kind="ExternalOutput").ap()
    nso_d = nc.dram_tensor("state_out", [2, N, D], F32, kind="ExternalOutput").ap()

    with tile.TileContext(nc) as tc, ExitStack() as ctx:
        const_p = ctx.enter_context(tc.tile_pool(name="const", bufs=1))
        qr_p = ctx.enter_context(tc.tile_pool(name="qr", bufs=1))
        rin_p = ctx.enter_context(tc.tile_pool(name="rin", bufs=3))
        rtmp_p = ctx.enter_context(tc.tile_pool(name="rtmp", bufs=2))
        st_p = ctx.enter_context(tc.tile_pool(name="st", bufs=2))
        qnt_p = ctx.enter_context(tc.tile_pool(name="qnt", bufs=3))
        ns_p = ctx.enter_context(tc.tile_pool(name="ns", bufs=2))
        ms_p = ctx.enter_context(tc.tile_pool(name="ms", bufs=1))
        osb_p = ctx.enter_context(tc.tile_pool(name="osb", bufs=1))
        pacc_p = ctx.enter_context(
            tc.tile_pool(name="pacc", bufs=1, space="PSUM")
        )
        ptr_p = ctx.enter_context(tc.tile_pool(name="ptr", bufs=2, space="PSUM"))
        pch_p = ctx.enter_context(tc.tile_pool(name="pch", bufs=1, space="PSUM"))

        # --- constants ---
        mask_sb = const_p.tile([128, 128], F32, tag="mask")
        nc.sync.dma_start(out=mask_sb[:], in_=mask_d)
        ident_sb = const_p.tile([128, 128], BF16, tag="ident")
        nc.sync.dma_start(out=ident_sb[:], in_=id_d)
        vbf = const_p.tile([128, 4 * D], BF16, tag="vbf")
        nc.gpsimd.dma_start(
            out=vbf.rearrange("q (a d) -> q a d", a=4),
            in_=v_d.rearrange("(a p) d -> p a d", p=128)
        )
        v2bf = const_p.tile([128, 4 * D], BF16, tag="v2bf")
        nc.vector.tensor_scalar_mul(v2bf[:], vbf[:], SCALE)

        # --- persistent QR tiles (bf16, [t, n] layout), 2 pairs x 4 t-tiles ---
        qr = [
            [qr_p.tile([128, N], BF16, tag=f"qr{p}{tt}", name=f"qr{p}{tt}") for tt in range(TTN)]
            for p in range(2)
        ]

        # --- phase R: rope both pairs, streaming cos/sin once ---
        for c8 in range(NC8):
            for tt in range(TTN):
                cosb = rin_p.tile([128, C], BF16, tag="cosb")
                nc.gpsimd.dma_start(
                    out=cosb[:],
                    in_=cos_d[tt * 128 : (tt + 1) * 128, c8 * C : (c8 + 1) * C],
                )
                sinb = rin_p.tile([128, C], BF16, tag="sinb")
                nc.gpsimd.dma_start(
                    out=sinb[:],
                    in_=sin_d[tt * 128 : (tt + 1) * 128, c8 * C : (c8 + 1) * C],
                )
                for p in range(2):
                    qb = rin_p.tile([128, C], BF16, tag="qb")
                    nc.gpsimd.dma_start(
                        out=qb[:],
                        in_=q_d[
                            p, tt * 128 : (tt + 1) * 128, c8 * C : (c8 + 1) * C
                        ],
                    )
                    rot = rtmp_p.tile([128, C], BF16, tag="rot")
                    qb3 = qb.rearrange("q (n two) -> q n two", two=2)
                    rot3 = rot.rearrange("q (n two) -> q n two", two=2)
                    nc.scalar.mul(rot3[:, :, 0], qb3[:, :, 1], -1.0)
                    nc.scalar.copy(rot3[:, :, 1], qb3[:, :, 0])
                    qrs = qr[p][tt][:, c8 * C : (c8 + 1) * C]
                    nc.vector.tensor_tensor(
                        out=qrs, in0=qb[:], in1=cosb[:], op=AluOpType.mult
                    )
                    m2 = rtmp_p.tile([128, C], BF16, tag="m2")
                    nc.vector.tensor_tensor(
                        out=m2[:], in0=rot[:], in1=sinb[:], op=AluOpType.mult
                    )
                    nc.vector.tensor_tensor(
                        out=qrs, in0=qrs, in1=m2[:], op=AluOpType.add
                    )

        # --- phase L: fused n-tile loop, one pair at a time ---
        for p in range(2):
            S0 = pacc_p.tile([128, 512], F32, tag="S0")
            S13 = pacc_p.tile([128, 512], F32, tag="S13")
            S2 = pacc_p.tile([128, 256], F32, tag="S2")
            OUT01 = pacc_p.tile([128, 512], F32, tag="OUT01")
            OUT23 = pacc_p.tile([128, 512], F32, tag="OUT23")
            out_view = [
                OUT01[:, 0:256],
                OUT01[:, 256:512],
                OUT23[:, 0:256],
                OUT23[:, 256:512],
            ]

            st8 = None
            ns8 = None
            for nt in range(NT):
                a = nt % SB
                if a == 0:
                    nt0 = nt
                    st8 = st_p.tile([128, SB * D], BF16, tag="st8")
                    nc.gpsimd.dma_start(
                        out=st8.rearrange("q (a d) -> q a d", a=SB),
                        in_=st_d[
                            p, nt0 * 128 : (nt0 + SB) * 128, :
                        ].rearrange("(a q) d -> q a d", q=128),
                    )
                    ns8 = ns_p.tile([128, SB * D], F32, tag="ns8")
                st_tile = st8[:, a * D : (a + 1) * D]

                # transpose 4 blocks of QR[:, nt] into one [128 n, 512 t] slab
                TR = ptr_p.tile([128, 512], BF16, tag="tr")
                for tt in range(TTN):
                    nc.tensor.transpose(
                        TR[:, tt * 128 : (tt + 1) * 128],
                        qr[p][tt][:, nt * 128 : (nt + 1) * 128],
                        ident_sb[:],
                    )
                qnt = qnt_p.tile([128, 512], BF16, tag="qnt")
                nc.scalar.copy(qnt[:], TR[:])

                # scores (QR @ QR^T): packed psum banks, lower-left skipped
                first = nt == 0
                last = nt == NT - 1
                nc.tensor.matmul(
                    S0[:, 0:512], qnt[:, 0:128], qnt[:, 0:512],
                    start=first, stop=last,
                )
                nc.tensor.matmul(
                    OUT01[:, 0:256], qnt[:, 0:128], st_tile,
                    start=first, stop=False,
                )
                nc.tensor.matmul(
                    S13[:, 0:384], qnt[:, 128:256], qnt[:, 128:512],
                    start=first, stop=last,
                )
                nc.tensor.matmul(
                    OUT01[:, 256:512], qnt[:, 128:256], st_tile,
                    start=first, stop=False,
                )
                nc.tensor.matmul(
                    S2[:, 0:256], qnt[:, 256:384], qnt[:, 256:512],
                    start=first, stop=last,
                )
                nc.tensor.matmul(
                    OUT23[:, 0:256], qnt[:, 256:384], st_tile,
                    start=first, stop=False,
                )
                nc.tensor.matmul(
                    S13[:, 384:512], qnt[:, 384:512], qnt[:, 384:512],
                    start=first, stop=last,
                )
                nc.tensor.matmul(
                    OUT23[:, 256:512], qnt[:, 384:512], st_tile,
                    start=first, stop=False,
                )

                # chunk state: QR^T @ V' accumulated + identity-matmul state add
                CH = pch_p.tile([128, 256], F32, tag="ch")
                for tt in range(TTN):
                    nc.tensor.matmul(
                        CH[:],
                        qr[p][tt][:, nt * 128 : (nt + 1) * 128],
                        v2bf[:, tt * D : (tt + 1) * D],
                        start=(tt == 0),
                        stop=False,
                    )
                nc.tensor.matmul(
                    CH[:], ident_sb[:], st_tile, start=False, stop=True
                )
                nc.scalar.copy(ns8[:, a * D : (a + 1) * D], CH[:])
                if a == SB - 1:
                    nc.sync.dma_start(
                        out=nso_d[
                            p, nt0 * 128 : (nt0 + SB) * 128, :
                        ].rearrange("(a q) d -> q a d", q=128),
                        in_=ns8.rearrange("q (a d) -> q a d", a=SB),
                    )

            # --- tail: masked scores -> SBUF bf16, then S @ V into OUT psum ---
            ms = [ms_p.tile([128, 512], BF16, tag=f"ms{jt}", name=f"ms{jt}") for jt in range(4)]
            # (j, i) views of the packed score banks, i-range [jt*128, 512)
            nc.vector.tensor_tensor(
                out=ms[0][:, 0:128], in0=S0[:, 0:128], in1=mask_sb[:],
                op=AluOpType.mult,
            )
            nc.scalar.mul(ms[0][:, 128:512], S0[:, 128:512], SCALE)
            nc.vector.tensor_tensor(
                out=ms[1][:, 128:256], in0=S13[:, 0:128], in1=mask_sb[:],
                op=AluOpType.mult,
            )
            nc.scalar.mul(ms[1][:, 256:512], S13[:, 128:384], SCALE)
            nc.vector.tensor_tensor(
                out=ms[2][:, 256:384], in0=S2[:, 0:128], in1=mask_sb[:],
                op=AluOpType.mult,
            )
            nc.scalar.mul(ms[2][:, 384:512], S2[:, 128:256], SCALE)
            nc.vector.tensor_tensor(
                out=ms[3][:, 384:512], in0=S13[:, 384:512], in1=mask_sb[:],
                op=AluOpType.mult,
            )

            for it in range(4):
                for jt in range(it + 1):
                    nc.tensor.matmul(
                        out_view[it],
                        ms[jt][:, it * 128 : (it + 1) * 128],
                        vbf[:, jt * D : (jt + 1) * D],
                        start=False,
                        stop=(jt == it),
                    )

            osb = osb_p.tile([128, 4 * D], F32, tag="osb")
            nc.scalar.copy(osb[:, 0:512], OUT01[:])
            nc.scalar.copy(osb[:, 512:1024], OUT23[:])
            nc.sync.dma_start(
                out=out_d[p].rearrange("(a q) d -> q a d", q=128),
                in_=osb.rearrange("q (a d) -> q a d", a=4)
            )

    nc.compile()
    return nc


def _get_nc():
    if "nc" not in _CACHE:
        _CACHE["nc"] = _build()
    return _CACHE["nc"]


def prep_in_maps(Q, V, state, cos, sin):
    mask = (SCALE * np.triu(np.ones((128, 128), np.float32), k=1)).astype(
        np.float32
    )
    ident = np.eye(128, dtype=ml_dtypes.bfloat16)

    Q = np.asarray(Q, np.float32)
    V = np.asarray(V, np.float32)
    state = np.asarray(state, np.float32)
    cos = np.ascontiguousarray(np.asarray(cos, np.float32)[:T])
    sin = np.ascontiguousarray(np.asarray(sin, np.float32)[:T])

    in_maps = []
    for c in range(NCORES):
        b = (2 * c) // NH
        h0 = (2 * c) % NH
        in_maps.append(
            {
                "q": np.ascontiguousarray(Q[b, h0 : h0 + 2]),
                "v": np.ascontiguousarray(V[b, 0]),
                "state_in": np.ascontiguousarray(state[b, h0 : h0 + 2]),
                "cos": cos,
                "sin": sin,
                "mask": mask,
                "ident": ident,
            }
        )
    return in_maps


def kernel(Q, V, state, cos, sin):
    from concourse.bass_utils import run_bass_kernel_spmd

    nc = _get_nc()
    in_maps = prep_in_maps(Q, V, state, cos, sin)
    res = run_bass_kernel_spmd(nc, in_maps, core_ids=list(range(NCORES)))
    _CACHE["last_result"] = res

    out = np.empty((B, NH, T, D), np.float32)
    new_state = np.empty((B, NH, N, D), np.float32)
    for c in range(NCORES):
        b = (2 * c) // NH
        h0 = (2 * c) % NH
        out[b, h0 : h0 + 2] = res.results[c]["out"]
        new_state[b, h0 : h0 + 2] = res.results[c]["state_out"]
    return out, new_state


# revision 31
# speedup vs baseline: 1.2802x; 1.2802x over previous
"""Trainium2 Bass kernel for nn_Attention_55645596287359 (sparse_attention).

Math per (b, h) pair (T=512, N=8192, D=256):
    QR   = rope(Q)                                   # interleaved-pair rope
    S    = (QR @ QR^T) * SCALE, strictly-causal-masked (j < i)
    out  = S @ V + QR @ state
    newS = state + (QR^T @ V) * SCALE

Sharding: 16 (b, h) pairs over 8 cores, 2 pairs per core (pure batch dims,
no cross-core communication).  Each core streams cos/sin once, ropes both
pairs into SBUF-resident bf16 QR tiles ([t, n] layout), then runs a fused
n-tile loop per pair: PE-transpose to get QR^T slabs, score accumulation,
out accumulation (QR @ state), and chunk-state (QR^T @ V') with the state
addition folded in as an identity matmul.  f32 in HBM, bf16 on the
TensorEngine, f32 PSUM accumulation.
"""

import sys

sys.path.insert(0, "/opt/trn_rl_repo")

import numpy as np
import ml_dtypes

B, NH, T, N, D = 4, 4, 512, 8192, 256
SCALE = float(N) ** -0.5
NCORES = 8
C = 2048              # rope streaming chunk (free-dim elements)
NC8 = N // C          # 8 chunks
NT = N // 128         # 64 n-tiles
TTN = T // 128        # 4 t-tiles
SB = 4                # state/new-state DMA batch (n-tiles per DMA)

F32 = None
BF16 = None

_CACHE = {}


def _build():
    import concourse.bacc as bacc
    import concourse.mybir as mybir
    import concourse.tile as tile
    from concourse.alu_op_type import AluOpType
    from contextlib import ExitStack

    F32 = mybir.dt.float32
    BF16 = mybir.dt.bfloat16

    nc = bacc.Bacc("TRN2", target_bir_lowering=False, debug=False)

    q_d = nc.dram_tensor("q", [2, T, N], F32, kind="ExternalInput").ap()
    v_d = nc.dram_tensor("v", [T, D], F32, kind="ExternalInput").ap()
    st_d = nc.dram_tensor("state_in", [2, N, D], F32, kind="ExternalInput").ap()
    cos_d = nc.dram_tensor("cos", [T, N], F32, kind="ExternalInput").ap()
    sin_d = nc.dram_tensor("sin", [T, N], F32, kind="ExternalInput").ap()
    mask_d = nc.dram_tensor("mask", [128, 128], F32, kind="ExternalInput").ap()
    id_d = nc.dram_tensor("ident", [128, 128], BF16, kind="ExternalInput").ap()
    out_d = nc.dram_tensor("out", [2, T, D], F32, kind="ExternalOutput").ap()
    nso_d = nc.dram_tensor("state_out", [2, N, D], F32, kind="ExternalOutput").ap()

    with tile.TileContext(nc) as tc, ExitStack() as ctx:
        const_p = ctx.enter_context(tc.tile_pool(name="const", bufs=1))
        qr_p = ctx.enter_context(tc.tile_pool(name="qr", bufs=1))
        rin_p = ctx.enter_context(tc.tile_pool(name="rin", bufs=3))
        rtmp_p = ctx.enter_context(tc.tile_pool(name="rtmp", bufs=2))
        st_p = ctx.enter_context(tc.tile_pool(name="st", bufs=2))
        qnt_p = ctx.enter_context(tc.tile_pool(name="qnt", bufs=3))
        ns_p = ctx.enter_context(tc.tile_pool(name="ns", bufs=2))
        ms_p = ctx.enter_context(tc.tile_pool(name="ms", bufs=1))
        pacc_p = ctx.enter_context(
            tc.tile_pool(name="pacc", bufs=1, space="PSUM")
        )
        ptr_p = ctx.enter_context(tc.tile_pool(name="ptr", bufs=2, space="PSUM"))
        pch_p = ctx.enter_context(tc.tile_pool(name="pch", bufs=1, space="PSUM"))

        # --- constants ---
        mask_sb = const_p.tile([128, 128], F32, tag="mask")
        nc.sync.dma_start(out=mask_sb[:], in_=mask_d)
        ident_sb = const_p.tile([128, 128], BF16, tag="ident")
        nc.sync.dma_start(out=ident_sb[:], in_=id_d)
        vbf = const_p.tile([128, 4 * D], BF16, tag="vbf")
        nc.gpsimd.dma_start(
            out=vbf.rearrange("q (a d) -> q a d", a=4),
            in_=v_d.rearrange("(a p) d -> p a d", p=128)
        )
        v2bf = const_p.tile([128, 4 * D], BF16, tag="v2bf")
        nc.vector.tensor_scalar_mul(v2bf[:], vbf[:], SCALE)

        # --- persistent QR tiles (bf16, [t, n] layout), 2 pairs x 4 t-tiles ---
        qr = [
            [qr_p.tile([128, N], BF16, tag=f"qr{p}{tt}", name=f"qr{p}{tt}") for tt in range(TTN)]
            for p in range(2)
        ]

        # --- phase R: rope both pairs, streaming cos/sin once ---
        for co in range(0, N, C):
            cw = C
            for tt in range(TTN):
                cosb = rin_p.tile([128, C], BF16, tag="cosb")
                nc.gpsimd.dma_start(
                    out=cosb[:, 0:cw],
                    in_=cos_d[tt * 128 : (tt + 1) * 128, co : co + cw],
                )
                sinb = rin_p.tile([128, C], BF16, tag="sinb")
                nc.gpsimd.dma_start(
                    out=sinb[:, 0:cw],
                    in_=sin_d[tt * 128 : (tt + 1) * 128, co : co + cw],
                )
                for p in range(2):
                    qb = rin_p.tile([128, C], BF16, tag="qb")
                    nc.gpsimd.dma_start(
                        out=qb[:, 0:cw],
                        in_=q_d[
                            p, tt * 128 : (tt + 1) * 128, co : co + cw
                        ],
                    )
                    rot = rtmp_p.tile([128, C], BF16, tag="rot")
                    qb3 = qb[:, 0:cw].rearrange("q (n two) -> q n two", two=2)
                    rot3 = rot[:, 0:cw].rearrange("q (n two) -> q n two", two=2)
                    if (tt + p) % 2 == 0:
                        nc.vector.tensor_scalar_mul(
                            rot3[:, :, 0], qb3[:, :, 1], -1.0
                        )
                        nc.vector.tensor_copy(rot3[:, :, 1], qb3[:, :, 0])
                    else:
                        nc.scalar.mul(rot3[:, :, 0], qb3[:, :, 1], -1.0)
                        nc.scalar.copy(rot3[:, :, 1], qb3[:, :, 0])
                    qrs = qr[p][tt][:, co : co + cw]
                    nc.vector.tensor_tensor(
                        out=qrs, in0=qb[:, 0:cw], in1=cosb[:, 0:cw],
                        op=AluOpType.mult,
                    )
                    m2 = rtmp_p.tile([128, C], BF16, tag="m2")
                    nc.vector.tensor_tensor(
                        out=m2[:, 0:cw], in0=rot[:, 0:cw], in1=sinb[:, 0:cw],
                        op=AluOpType.mult,
                    )
                    nc.vector.tensor_tensor(
                        out=qrs, in0=qrs, in1=m2[:, 0:cw], op=AluOpType.add
                    )

        # --- phase L: fused n-tile loop, one pair at a time ---
        for p in range(2):
            S0 = pacc_p.tile([128, 512], F32, tag="S0")
            S13 = pacc_p.tile([128, 512], F32, tag="S13")
            S2 = pacc_p.tile([128, 256], F32, tag="S2")
            OUT01 = pacc_p.tile([128, 512], F32, tag="OUT01")
            OUT23 = pacc_p.tile([128, 512], F32, tag="OUT23")
            out_view = [
                OUT01[:, 0:256],
                OUT01[:, 256:512],
                OUT23[:, 0:256],
                OUT23[:, 256:512],
            ]

            st8 = None
            ns8 = None
            for ntp in range(NT // 2):
                nts = (2 * ntp, 2 * ntp + 1)
                if nts[0] % SB == 0:
                    nt0 = nts[0]
                    st8 = st_p.tile([128, SB * D], BF16, tag="st8")
                    nc.gpsimd.dma_start(
                        out=st8.rearrange("q (a d) -> q a d", a=SB),
                        in_=st_d[
                            p, nt0 * 128 : (nt0 + SB) * 128, :
                        ].rearrange("(a q) d -> q a d", q=128),
                    )
                    ns8 = ns_p.tile([128, SB * D], F32, tag="ns8")

                # transpose 2x4 blocks of QR into one [128 n, 2*512 t] slab
                TR = ptr_p.tile([128, 1024], BF16, tag="tr")
                for h, nt in enumerate(nts):
                    for tt in range(TTN):
                        nc.tensor.transpose(
                            TR[:, h * 512 + tt * 128 : h * 512 + (tt + 1) * 128],
                            qr[p][tt][:, nt * 128 : (nt + 1) * 128],
                            ident_sb[:],
                        )

                # chunk state (qnt-independent -> PE gap filler):
                # QR^T @ V' accumulated + identity-matmul state add.
                # One [128,512] psum bank holds both nts' chunks.
                CH = pch_p.tile([128, 512], F32, tag="ch", name=f"ch{ntp}")
                for h, nt in enumerate(nts):
                    a = nt % SB
                    st_tile = st8[:, a * D : (a + 1) * D]
                    chv = CH[:, h * D : (h + 1) * D]
                    for tt in range(TTN):
                        nc.tensor.matmul(
                            chv,
                            qr[p][tt][:, nt * 128 : (nt + 1) * 128],
                            v2bf[:, tt * D : (tt + 1) * D],
                            start=(tt == 0),
                            stop=False,
                        )
                    nc.tensor.matmul(
                        chv, ident_sb[:], st_tile, start=False, stop=True
                    )
                a0 = nts[0] % SB
                if p == 0:
                    nc.scalar.copy(ns8[:, a0 * D : (a0 + 2) * D], CH[:])
                else:
                    nc.vector.tensor_copy(ns8[:, a0 * D : (a0 + 2) * D], CH[:])

                qnt2 = qnt_p.tile([128, 1024], BF16, tag="qnt")
                if p == 0:
                    nc.scalar.copy(qnt2[:], TR[:])
                else:
                    nc.vector.tensor_copy(qnt2[:], TR[:])

                # scores (QR @ QR^T, packed psum banks) + out accumulation
                for h, nt in enumerate(nts):
                    a = nt % SB
                    st_tile = st8[:, a * D : (a + 1) * D]
                    qnt = qnt2[:, h * 512 : (h + 1) * 512]
                    first = nt == 0
                    last = nt == NT - 1
                    nc.tensor.matmul(
                        S0[:, 0:512], qnt[:, 0:128], qnt[:, 0:512],
                        start=first, stop=last,
                    )
                    nc.tensor.matmul(
                        OUT01[:, 0:256], qnt[:, 0:128], st_tile,
                        start=first, stop=False,
                    )
                    nc.tensor.matmul(
                        S13[:, 0:384], qnt[:, 128:256], qnt[:, 128:512],
                        start=first, stop=last,
                    )
                    nc.tensor.matmul(
                        OUT01[:, 256:512], qnt[:, 128:256], st_tile,
                        start=first, stop=False,
                    )
                    nc.tensor.matmul(
                        S2[:, 0:256], qnt[:, 256:384], qnt[:, 256:512],
                        start=first, stop=last,
                    )
                    nc.tensor.matmul(
                        OUT23[:, 0:256], qnt[:, 256:384], st_tile,
                        start=first, stop=False,
                    )
                    nc.tensor.matmul(
                        S13[:, 384:512], qnt[:, 384:512], qnt[:, 384:512],
                        start=first, stop=last,
                    )
                    nc.tensor.matmul(
                        OUT23[:, 256:512], qnt[:, 384:512], st_tile,
                        start=first, stop=False,
                    )

                if nts[1] % SB == SB - 1:
                    nc.sync.dma_start(
                        out=nso_d[
                            p, nt0 * 128 : (nt0 + SB) * 128, :
                        ].rearrange("(a q) d -> q a d", q=128),
                        in_=ns8.rearrange("q (a d) -> q a d", a=SB),
                    )

            # --- tail: masked scores -> SBUF bf16, then S @ V into OUT psum ---
            ms_full = [
                ms_p.tile([128, 512 - 128 * jt], BF16, tag=f"ms{jt}", name=f"ms{jt}")
                for jt in range(4)
            ]

            def msv(jt, lo, hi):
                return ms_full[jt][:, lo - 128 * jt : hi - 128 * jt]
            # (j, i) views of the packed score banks, i-range [jt*128, 512)
            nc.vector.tensor_tensor(
                out=msv(0, 0, 128), in0=S0[:, 0:128], in1=mask_sb[:],
                op=AluOpType.mult,
            )
            nc.scalar.mul(msv(0, 128, 512), S0[:, 128:512], SCALE)
            nc.vector.tensor_tensor(
                out=msv(1, 128, 256), in0=S13[:, 0:128], in1=mask_sb[:],
                op=AluOpType.mult,
            )
            nc.scalar.mul(msv(1, 256, 512), S13[:, 128:384], SCALE)
            nc.vector.tensor_tensor(
                out=msv(2, 256, 384), in0=S2[:, 0:128], in1=mask_sb[:],
                op=AluOpType.mult,
            )
            nc.scalar.mul(msv(2, 384, 512), S2[:, 128:256], SCALE)
            nc.vector.tensor_tensor(
                out=msv(3, 384, 512), in0=S13[:, 384:512], in1=mask_sb[:],
                op=AluOpType.mult,
            )

            for it in range(4):
                for jt in range(it + 1):
                    nc.tensor.matmul(
                        out_view[it],
                        msv(jt, it * 128, (it + 1) * 128),
                        vbf[:, jt * D : (jt + 1) * D],
                        start=False,
                        stop=(jt == it),
                    )

            osb = ns_p.tile([128, 4 * D], F32, tag="ns8", name="osb")
            nc.scalar.copy(osb[:, 0:512], OUT01[:])
            nc.scalar.copy(osb[:, 512:1024], OUT23[:])
            nc.sync.dma_start(
                out=out_d[p].rearrange("(a q) d -> q a d", q=128),
                in_=osb.rearrange("q (a d) -> q a d", a=4)
            )

    nc.compile()
    return nc


def _get_nc():
    if "nc" not in _CACHE:
        _CACHE["nc"] = _build()
    return _CACHE["nc"]


def prep_in_maps(Q, V, state, cos, sin):
    mask = (SCALE * np.triu(np.ones((128, 128), np.float32), k=1)).astype(
        np.float32
    )
    ident = np.eye(128, dtype=ml_dtypes.bfloat16)

    Q = np.asarray(Q, np.float32)
    V = np.asarray(V, np.float32)
    state = np.asarray(state, np.float32)
    cos = np.ascontiguousarray(np.asarray(cos, np.float32)[:T])
    sin = np.ascontiguousarray(np.asarray(sin, np.float32)[:T])

    in_maps = []
    for c in range(NCORES):
        b = (2 * c) // NH
        h0 = (2 * c) % NH
        in_maps.append(
            {
                "q": np.ascontiguousarray(Q[b, h0 : h0 + 2]),
                "v": np.ascontiguousarray(V[b, 0]),
                "state_in": np.ascontiguousarray(state[b, h0 : h0 + 2]),
                "cos": cos,
                "sin": sin,
                "mask": mask,
                "ident": ident,
            }
        )
    return in_maps


def kernel(Q, V, state, cos, sin):
    from concourse.bass_utils import run_bass_kernel_spmd

    nc = _get_nc()
    in_maps = prep_in_maps(Q, V, state, cos, sin)
    res = run_bass_kernel_spmd(nc, in_maps, core_ids=list(range(NCORES)))
    _CACHE["last_result"] = res

    out = np.empty((B, NH, T, D), np.float32)
    new_state = np.empty((B, NH, N, D), np.float32)
    for c in range(NCORES):
        b = (2 * c) // NH
        h0 = (2 * c) % NH
        out[b, h0 : h0 + 2] = res.results[c]["out"]
        new_state[b, h0 : h0 + 2] = res.results[c]["state_out"]
    return out, new_state
